# revision 1
# baseline (speedup 1.0000x reference)
"""Trainium2 Bass kernel for nn_Decoder: 16-step GRU decoder with vocab-parallel
tensor sharding across 8 NeuronCores.

Sharding: vocab dim (V=32000, padded to 32768) split 4096/core. Per step:
  - logits shard (Vs,B) = w_out_shard @ h1  (weight-stationary, bf16)
  - softmax over batch axis = free-axis reduction, core-local
  - x partial = probs_shard @ w_in_shard.T  -> AllReduce (B*H fp32)
  - GRU hidden dim H=1024 split 128/core; AllGather h0, h1 (bf16)
All activations live in transposed (feature, batch) layout so softmax output
probsT directly feeds the next matmul as the moving operand.
"""
import numpy as np
import ml_dtypes

import concourse.bass as bass
import concourse.mybir as mybir
import concourse.tile as tile
from concourse import bacc
from concourse import bass_utils

F32 = mybir.dt.float32
BF16 = mybir.dt.bfloat16
AF = mybir.ActivationFunctionType

B = 128
H = 1024
V = 32000
T = 16
BOS = 1
NC = 8
VS = 4096          # per-core padded vocab
VT = VS // 128     # 32 vocab tiles per core
KH = H // 128      # 8 H k-tiles
VPAD = NC * VS     # 32768


def build_nc(steps=T, n_cores=NC, enable_asserts=False):
    nc = bacc.Bacc("TRN2", target_bir_lowering=False, debug=False,
                   num_devices=n_cores, enable_asserts=enable_asserts)
    rg = [list(range(n_cores))]

    # ---- DRAM I/O ----
    d_woutT = nc.dram_tensor("woutT", [128, KH, VS], BF16, kind="ExternalInput").ap()
    d_bout = nc.dram_tensor("bout", [128, VT], F32, kind="ExternalInput").ap()
    d_winT = nc.dram_tensor("winT", [128, VT, H], BF16, kind="ExternalInput").ap()
    d_g = {}
    for nm in ("gih0", "ghh0", "gih1", "ghh1"):
        d_g[nm] = nc.dram_tensor(nm, [128, KH, 384], BF16, kind="ExternalInput").ap()
    d_brz0 = nc.dram_tensor("brz0", [128, 2], F32, kind="ExternalInput").ap()
    d_bn0 = nc.dram_tensor("bn0", [128, 2], F32, kind="ExternalInput").ap()  # [:,0]=b_in_n eff, [:,1]=b_hn
    d_brz1 = nc.dram_tensor("brz1", [128, 2], F32, kind="ExternalInput").ap()
    d_bn1 = nc.dram_tensor("bn1", [128, 2], F32, kind="ExternalInput").ap()
    d_x0T = nc.dram_tensor("x0T", [128, KH, B], BF16, kind="ExternalInput").ap()
    d_h0f = nc.dram_tensor("h0f", [128, KH, B], F32, kind="ExternalInput").ap()
    d_h1f = nc.dram_tensor("h1f", [128, KH, B], F32, kind="ExternalInput").ap()
    d_h0own = nc.dram_tensor("h0own", [128, B], F32, kind="ExternalInput").ap()
    d_h1own = nc.dram_tensor("h1own", [128, B], F32, kind="ExternalInput").ap()
    d_out = nc.dram_tensor("logits", [steps, VT, 128, B], F32, kind="ExternalOutput").ap()

    with tile.TileContext(nc) as tc:
        with tc.tile_pool(name="wpool", bufs=1) as wpool, \
             tc.tile_pool(name="state", bufs=1) as state, \
             tc.tile_pool(name="sb", bufs=3) as sb, \
             tc.tile_pool(name="ps", bufs=1, space="PSUM") as ps, \
             tc.tile_pool(name="dram", bufs=2, space="DRAM") as dram:

            # ---- load weights to SBUF (resident) ----
            wout_sb = wpool.tile([128, KH, VS], BF16)
            for k in range(KH):
                nc.sync.dma_start(wout_sb[:, k, :], d_woutT[:, k, :])
            win_sb = wpool.tile([128, VT, H], BF16)
            for vo in range(VT):
                nc.sync.dma_start(win_sb[:, vo, :], d_winT[:, vo, :])
            g_sb = {}
            for nm in ("gih0", "ghh0", "gih1", "ghh1"):
                t_ = wpool.tile([128, KH, 384], BF16, name=nm + "_sb")
                nc.sync.dma_start(t_[:], d_g[nm][:])
                g_sb[nm] = t_
            bout_sb = wpool.tile([128, VT], F32)
            nc.sync.dma_start(bout_sb[:], d_bout[:])
            brz = [wpool.tile([128, 2], F32, name=f"brz{l}_sb") for l in range(2)]
            bn = [wpool.tile([128, 2], F32, name=f"bn{l}_sb") for l in range(2)]
            nc.sync.dma_start(brz[0][:], d_brz0[:])
            nc.sync.dma_start(brz[1][:], d_brz1[:])
            nc.sync.dma_start(bn[0][:], d_bn0[:])
            nc.sync.dma_start(bn[1][:], d_bn1[:])

            x0_sb = state.tile([128, KH, B], BF16)
            nc.sync.dma_start(x0_sb[:], d_x0T[:])

            # ---- hidden state init ----
            hf = []   # full hidden, bf16 [128, KH, B]
            hown = []  # own chunk fp32 [128, B]
            for l, (dfull, downn) in enumerate(((d_h0f, d_h0own), (d_h1f, d_h1own))):
                tmp = sb.tile([128, KH, B], F32, tag="hinit", name=f"hinit{l}")
                nc.sync.dma_start(tmp[:], dfull[:])
                fb = state.tile([128, KH, B], BF16, name=f"h{l}fb")
                nc.vector.tensor_copy(out=fb[:], in_=tmp[:])
                hf.append(fb)
                own = state.tile([128, B], F32, name=f"h{l}own")
                nc.sync.dma_start(own[:], downn[:])
                hown.append(own)

            xf32 = state.tile([128, KH, B], F32)
            xbf = state.tile([128, KH, B], BF16)

            def gru_layer(l, t, x_rhs_tiles, gih, ghh):
                """x_rhs_tiles: list of KH rhs APs (128,B) bf16. Updates hown[l], returns nothing.
                Emits hh-dependent matmuls first so they can overlap the preceding collective."""
                grz = ps.tile([128, 256], F32, tag=f"rz{l}", name=f"rz{l}_{t}")
                gnn = ps.tile([128, 256], F32, tag=f"nn{l}", name=f"nn{l}_{t}")
                # r and z gates: accumulate hh first (ready early), then ih
                for g in range(2):
                    o = g * 128
                    for k in range(KH):
                        nc.tensor.matmul(grz[:, o:o + 128], ghh[:, k, o:o + 128],
                                         hf[l][:, k, :], start=(k == 0), stop=False)
                    for k in range(KH):
                        nc.tensor.matmul(grz[:, o:o + 128], gih[:, k, o:o + 128],
                                         x_rhs_tiles[k], start=False, stop=(k == KH - 1))
                # n gate: hn (hh) and in (ih) kept separate
                for k in range(KH):
                    nc.tensor.matmul(gnn[:, 128:256], ghh[:, k, 256:384],
                                     hf[l][:, k, :], start=(k == 0), stop=(k == KH - 1))
                for k in range(KH):
                    nc.tensor.matmul(gnn[:, 0:128], gih[:, k, 256:384],
                                     x_rhs_tiles[k], start=(k == 0), stop=(k == KH - 1))
                # elementwise
                r = sb.tile([128, B], F32, tag="ew_r", name=f"r{l}_{t}")
                nc.scalar.activation(r[:], grz[:, 0:128], AF.Sigmoid, bias=brz[l][:, 0:1])
                z = sb.tile([128, B], F32, tag="ew_z", name=f"z{l}_{t}")
                nc.scalar.activation(z[:], grz[:, 128:256], AF.Sigmoid, bias=brz[l][:, 1:2])
                hn = sb.tile([128, B], F32, tag="ew_hn", name=f"hn{l}_{t}")
                nc.scalar.activation(hn[:], gnn[:, 128:256], AF.Identity, bias=bn[l][:, 1:2])
                rhn = sb.tile([128, B], F32, tag="ew_rhn", name=f"rhn{l}_{t}")
                nc.vector.tensor_mul(out=rhn[:], in0=r[:], in1=hn[:])
                pre = sb.tile([128, B], F32, tag="ew_pre", name=f"pre{l}_{t}")
                nc.vector.tensor_add(out=pre[:], in0=rhn[:], in1=gnn[:, 0:128])
                n = sb.tile([128, B], F32, tag="ew_n", name=f"n{l}_{t}")
                nc.scalar.activation(n[:], pre[:], AF.Tanh, bias=bn[l][:, 0:1])
                s = sb.tile([128, B], F32, tag="ew_s", name=f"s{l}_{t}")
                nc.vector.tensor_sub(out=s[:], in0=hown[l][:], in1=n[:])
                zs = sb.tile([128, B], F32, tag="ew_zs", name=f"zs{l}_{t}")
                nc.vector.tensor_mul(out=zs[:], in0=z[:], in1=s[:])
                nc.vector.tensor_add(out=hown[l][:], in0=n[:], in1=zs[:])
                # cast + allgather
                hb = sb.tile([128, B], BF16, tag="agc", name=f"agc{l}_{t}")
                nc.vector.tensor_copy(out=hb[:], in_=hown[l][:])
                agin = dram.tile([128, B], BF16, tag=f"agin{l}", name=f"agin{l}_{t}")
                agout = dram.tile([n_cores * 128, B], BF16, tag=f"agout{l}",
                                  name=f"agout{l}_{t}")
                nc.sync.dma_start(agin[:], hb[:])
                nc.gpsimd.collective_compute(
                    "AllGather", mybir.AluOpType.bypass, replica_groups=rg,
                    ins=[agin.opt()], outs=[agout.opt()])
                for k in range(KH):
                    nc.sync.dma_start(hf[l][:, k, :], agout[k * 128:(k + 1) * 128, :])

            for t in range(steps):
                x_rhs = [x0_sb[:, k, :] for k in range(KH)] if t == 0 \
                    else [xbf[:, k, :] for k in range(KH)]
                gru_layer(0, t, x_rhs, g_sb["gih0"], g_sb["ghh0"])
                gru_layer(1, t, [hf[0][:, k, :] for k in range(KH)],
                          g_sb["gih1"], g_sb["ghh1"])

                # ---- logits + softmax ----
                last = (t == steps - 1)
                probs = []
                if not last:
                    sums = sb.tile([128, VT], F32, tag="sums", name=f"sums_{t}")
                    recs = sb.tile([128, VT], F32, tag="recs", name=f"recs_{t}")
                for j in range(VT):
                    lg = ps.tile([128, B], F32, tag="lg", bufs=2, name=f"lg_{t}_{j}")
                    for k in range(KH):
                        nc.tensor.matmul(lg[:], wout_sb[:, k, j * 128:(j + 1) * 128],
                                         hf[1][:, k, :], start=(k == 0), stop=(k == KH - 1))
                    lout = sb.tile([128, B], F32, tag="lout", bufs=4, name=f"lout_{t}_{j}")
                    nc.vector.tensor_scalar_add(lout[:], lg[:], bout_sb[:, j:j + 1])
                    nc.sync.dma_start(d_out[t, j], lout[:])
                    if not last:
                        pe = sb.tile([128, B], BF16, tag="probs", bufs=VT,
                                     name=f"probs_{t}_{j}")
                        nc.scalar.activation(pe[:], lg[:], AF.Exp,
                                             bias=bout_sb[:, j:j + 1],
                                             accum_out=sums[:, j:j + 1])
                        nc.vector.reciprocal(recs[:, j:j + 1], sums[:, j:j + 1])
                        nc.vector.tensor_scalar_mul(pe[:], pe[:], recs[:, j:j + 1])
                        probs.append(pe)

                if not last:
                    # ---- x partial + allreduce ----
                    arin = dram.tile([KH, 128, B], F32, tag="arin", name=f"arin_{t}")
                    arout = dram.tile([KH, 128, B], F32, tag="arout", name=f"arout_{t}")
                    for m in range(KH):
                        xp = ps.tile([128, B], F32, tag="xp", bufs=2, name=f"xp_{t}_{m}")
                        for vo in range(VT):
                            nc.tensor.matmul(xp[:], win_sb[:, vo, m * 128:(m + 1) * 128],
                                             probs[vo][:], start=(vo == 0),
                                             stop=(vo == VT - 1))
                        xps = sb.tile([128, B], F32, tag="xps", bufs=2,
                                      name=f"xps_{t}_{m}")
                        nc.vector.tensor_copy(out=xps[:], in_=xp[:])
                        nc.sync.dma_start(arin[m], xps[:])
                    nc.gpsimd.collective_compute(
                        "AllReduce", mybir.AluOpType.add, replica_groups=rg,
                        ins=[arin.opt()], outs=[arout.opt()])
                    for m in range(KH):
                        nc.sync.dma_start(xf32[:, m, :], arout[m])
                        nc.vector.tensor_copy(out=xbf[:, m, :], in_=xf32[:, m, :])

    nc.compile()
    return nc


# ---------------- host side ----------------

def _prep_core_inputs(c, hidden, w_in, b_in, W_ih0, W_hh0, b_ih0, b_hh0,
                      W_ih1, W_hh1, b_ih1, b_hh1, w_out, b_out):
    bf = ml_dtypes.bfloat16
    w_inT_pad = np.zeros((VPAD, H), np.float32)
    w_inT_pad[:V] = w_in.T
    w_outT_pad = np.zeros((H, VPAD), np.float32)
    w_outT_pad[:, :V] = w_out.T
    b_out_pad = np.zeros(VPAD, np.float32)
    b_out_pad[:V] = b_out

    d = {}
    wv = w_outT_pad[:, c * VS:(c + 1) * VS]              # (H, VS)
    d["woutT"] = np.ascontiguousarray(
        wv.reshape(KH, 128, VS).transpose(1, 0, 2)).astype(bf)
    d["bout"] = np.ascontiguousarray(
        b_out_pad[c * VS:(c + 1) * VS].reshape(VT, 128).T)
    winv = w_inT_pad[c * VS:(c + 1) * VS, :]              # (VS, H)
    d["winT"] = np.ascontiguousarray(
        winv.reshape(VT, 128, H).transpose(1, 0, 2)).astype(bf)

    sel = np.concatenate([np.arange(c * 128, (c + 1) * 128) + g * H for g in range(3)])
    for nm, W in (("gih0", W_ih0), ("ghh0", W_hh0), ("gih1", W_ih1), ("ghh1", W_hh1)):
        Wsel = W[sel]                                     # (384, H)
        d[nm] = np.ascontiguousarray(
            Wsel.T.reshape(KH, 128, 384).transpose(1, 0, 2)).astype(bf)

    for l, (W_ih, b_ih, b_hh) in enumerate(((W_ih0, b_ih0, b_hh0),
                                            (W_ih1, b_ih1, b_hh1))):
        ih_eff = b_ih[sel].astype(np.float32)
        if l == 0:
            ih_eff = ih_eff + W_ih0[sel] @ b_in
        hh = b_hh[sel].astype(np.float32)
        d[f"brz{l}"] = np.stack([ih_eff[:128] + hh[:128],
                                 ih_eff[128:256] + hh[128:256]], axis=1)
        d[f"bn{l}"] = np.stack([ih_eff[256:384], hh[256:384]], axis=1)

    x0 = w_inT_pad[BOS]                                   # (H,) == w_in[:, BOS]
    d["x0T"] = np.ascontiguousarray(
        np.broadcast_to(x0.reshape(KH, 128).T[:, :, None], (128, KH, B))).astype(bf)

    for l in range(2):
        hT = hidden[l].T                                  # (H, B)
        d[f"h{l}f"] = np.ascontiguousarray(
            hT.reshape(KH, 128, B).transpose(1, 0, 2)).astype(np.float32)
        d[f"h{l}own"] = np.ascontiguousarray(
            hT[c * 128:(c + 1) * 128]).astype(np.float32)
    return {k: np.ascontiguousarray(v) for k, v in d.items()}


_NC_CACHE = {}


def _get_nc(steps=T):
    if steps not in _NC_CACHE:
        _NC_CACHE[steps] = build_nc(steps)
    return _NC_CACHE[steps]


def kernel(**inputs):
    nc = _get_nc(T)
    in_maps = [_prep_core_inputs(c, **inputs) for c in range(NC)]
    res = bass_utils.run_bass_kernel_spmd(nc, in_maps, core_ids=list(range(NC)))
    out_pad = np.zeros((T, B, VPAD), np.float32)
    for c in range(NC):
        o = res.results[c]["logits"]                      # (T, VT, 128, B)
        out_pad[:, :, c * VS:(c + 1) * VS] = o.transpose(0, 3, 1, 2).reshape(T, B, VS)
    return out_pad[:, :, :V]
